# revision 5
# baseline (speedup 1.0000x reference)
"""GCN layer (nn_GCN2) on 8 Trainium2 NeuronCores.

out = relu(segment_sum(w_e * (x@W)[src_e], dst) + b) + x

Strategy (uses (A@x)@W == A@(x@W) associativity):
  - dst-shard nodes across 8 cores (6250 rows each); x is an *input* so the
    gather table (bf16 copy of x) is replicated to every core — no halo
    exchange of computed tensors is needed.
  - per core: dma_gather (SWDGE, 4 queues) pulls the bf16 x-rows of each
    128-edge chunk into SBUF [128 edges, 512]; a one-hot scatter matmul
    S[128e,128dst]^T @ rows accumulates each 128-dst block in PSUM (f32).
    S = is_equal(iota, dst_local) * w built on VectorE.
  - agg block -> PE transpose -> GEMM with W (bf16, f32 accum) + bias via a
    rank-1 matmul -> relu (ScalarE) -> + x (residual, f32) -> out.
  - int16 gather indices only reach 32767, so the table is split lo/hi
    (25000 rows each) and each block's edges are bucketed by src half.
    Trailing -1 indices are trimmed by the Q7 ucode, so padded chunk slots
    cost no gather traffic.
"""
import numpy as np
import ml_dtypes

N_NODES = 50000
D = 512
N_CORES = 8
SHARD = N_NODES // N_CORES          # 6250
BLK = 128
N_BLK = (SHARD + BLK - 1) // BLK    # 49 (last block 106 rows)
HALF = N_NODES // 2                 # 25000
BF = ml_dtypes.bfloat16

_CACHE = {}


def _pack_idx(idx_flat):
    """[n*16] int16 -> [128, n] (16-partition column wrap, replicated 8x)."""
    n = idx_flat.shape[0]
    a = idx_flat.reshape(n // 16, 16).T.astype(np.int16)
    return np.tile(a, (8, 1))


def _prep_core(c, es, ed, ew, cpb):
    """Per-core edge metadata. Returns (srci [128, NB*2*cpb*8] int16,
    dcol [128, NB*2*cpb] bf16, wv [128, NB*2*cpb] bf16)."""
    cap = cpb * BLK
    lo, hi = c * SHARD, (c + 1) * SHARD
    m = (ed >= lo) & (ed < hi)
    src_c = es[m]
    dst_l = ed[m] - lo
    w_c = ew[m]
    blk = dst_l // BLK
    half = (src_c >= HALF).astype(np.int64)
    # order edges by (block, half); within that, arbitrary
    order = np.lexsort((half, blk))
    src_c, dst_l, w_c, blk, half = (
        src_c[order], dst_l[order], w_c[order], blk[order], half[order])
    key = blk * 2 + half
    counts = np.bincount(key, minlength=N_BLK * 2)
    starts = np.concatenate([[0], np.cumsum(counts)])

    srci_cols = []
    dcol = np.zeros((N_BLK * 2 * cpb, BLK), np.float32)
    wv = np.zeros((N_BLK * 2 * cpb, BLK), np.float32)
    for b in range(N_BLK):
        for h in (0, 1):
            k = b * 2 + h
            s, n = starts[k], counts[k]
            assert n <= cap
            # pad with index 0 (real row, finite data; zero weight in S)
            idx = np.zeros(cap, np.int16)
            sl = slice(s, s + n)
            idx[:n] = (src_c[sl] - h * HALF).astype(np.int16)
            srci_cols.append(_pack_idx(idx))
            # chunk j slot p = edge j*128+p
            base = k * cpb
            jj = np.arange(n) // BLK
            pp = np.arange(n) % BLK
            dcol[base + jj, pp] = (dst_l[sl] % BLK).astype(np.float32)
            wv[base + jj, pp] = w_c[sl]
    srci = np.concatenate(srci_cols, axis=1)
    return srci, dcol.T.copy().astype(BF), wv.T.copy().astype(BF)


def _build(cpb):
    import concourse.bass as bass
    import concourse.bacc as bacc
    import concourse.mybir as mybir
    import concourse.tile as tile

    BF16 = mybir.dt.bfloat16
    F32 = mybir.dt.float32
    I16 = mybir.dt.int16
    IC = cpb * 8  # idx cols per (block, half)

    nc = bacc.Bacc(None, target_bir_lowering=False, num_swdge_queues=4)
    xb_lo = nc.declare_dram_parameter("xb_lo", [HALF, D], BF16, isOutput=False)
    xb_hi = nc.declare_dram_parameter("xb_hi", [HALF, D], BF16, isOutput=False)
    w_in = nc.declare_dram_parameter("w_in", [D, D], BF16, isOutput=False)
    b_in = nc.declare_dram_parameter("b_in", [1, D], BF16, isOutput=False)
    ident_in = nc.declare_dram_parameter("ident", [128, 128], BF16, isOutput=False)
    iota_in = nc.declare_dram_parameter("iota", [128, cpb * BLK], BF16, isOutput=False)
    srci_in = nc.declare_dram_parameter("srci", [128, N_BLK * 2 * IC], I16, isOutput=False)
    dcol_in = nc.declare_dram_parameter("dcol", [128, N_BLK * 2 * cpb], BF16, isOutput=False)
    wv_in = nc.declare_dram_parameter("wv", [128, N_BLK * 2 * cpb], BF16, isOutput=False)
    xown_in = nc.declare_dram_parameter("xown", [SHARD, D], F32, isOutput=False)
    out = nc.declare_dram_parameter("out", [SHARD, D], F32, isOutput=True)

    with tile.TileContext(nc) as tc:
        with (
            tc.tile_pool(name="const", bufs=1) as constp,
            tc.tile_pool(name="meta", bufs=1) as metap,
            tc.tile_pool(name="stage", bufs=4) as stagep,
            tc.tile_pool(name="sbuild", bufs=4) as sbuildp,
            tc.tile_pool(name="aggsb", bufs=2) as aggsbp,
            tc.tile_pool(name="aggt", bufs=2) as aggtp,
            tc.tile_pool(name="xo", bufs=2) as xop,
            tc.tile_pool(name="res", bufs=2) as resp,
            tc.tile_pool(name="psA", bufs=2, space="PSUM") as psA,
            tc.tile_pool(name="psT", bufs=2, space="PSUM") as psT,
            tc.tile_pool(name="psO", bufs=2, space="PSUM") as psO,
        ):
            # constants / metadata
            wt = constp.tile([128, 4, D], BF16)      # W rows as 4 [128, 512]
            nc.sync.dma_start(out=wt[:], in_=w_in.rearrange("(q k) n -> k q n", q=4))
            bias = constp.tile([1, D], BF16)
            nc.sync.dma_start(out=bias[:], in_=b_in[:])
            ones1 = constp.tile([1, 128], BF16)
            nc.vector.memset(ones1[:], 1.0)
            ident = constp.tile([128, 128], BF16)
            nc.sync.dma_start(out=ident[:], in_=ident_in[:])
            iota = constp.tile([128, cpb * BLK], BF16)
            nc.sync.dma_start(out=iota[:], in_=iota_in[:])
            srci = metap.tile([128, N_BLK * 2 * IC], I16)
            nc.sync.dma_start(out=srci[:], in_=srci_in[:])
            dcol = metap.tile([128, N_BLK * 2 * cpb], BF16)
            nc.sync.dma_start(out=dcol[:], in_=dcol_in[:])
            wv = metap.tile([128, N_BLK * 2 * cpb], BF16)
            nc.sync.dma_start(out=wv[:], in_=wv_in[:])

            iota3 = iota[:].rearrange("p (c k) -> p c k", k=BLK)

            q = 0
            for blk in range(N_BLK):
                rows = min(BLK, SHARD - blk * BLK)
                stages = []
                for h, table in ((0, xb_lo), (1, xb_hi)):
                    k = blk * 2 + h
                    st = stagep.tile([128, cpb, D], BF16)
                    nc.gpsimd.dma_gather(
                        st[:], table[:], srci[:, k * IC:(k + 1) * IC],
                        cpb * BLK, cpb * BLK, D,
                        single_packet=False, queue_num=q % 4,
                    )
                    q += 1
                    # S = (iota == dcol) * w
                    S = sbuildp.tile([128, cpb, BLK], BF16)
                    csl = slice(k * cpb, (k + 1) * cpb)
                    nc.vector.tensor_tensor(
                        out=S[:], in0=iota3,
                        in1=dcol[:, csl][:, :, None].to_broadcast([128, cpb, BLK]),
                        op=mybir.AluOpType.is_equal,
                    )
                    nc.vector.tensor_tensor(
                        out=S[:], in0=S[:],
                        in1=wv[:, csl][:, :, None].to_broadcast([128, cpb, BLK]),
                        op=mybir.AluOpType.mult,
                    )
                    stages.append((st, S))

                ps_agg = psA.tile([128, D], F32, space="PSUM")
                n_mm = 2 * cpb
                mi = 0
                for st, S in stages:
                    for j in range(cpb):
                        nc.tensor.matmul(
                            out=ps_agg[:], lhsT=S[:, j, :], rhs=st[:, j, :],
                            start=(mi == 0), stop=(mi == n_mm - 1),
                        )
                        mi += 1

                agg_sb = aggsbp.tile([128, D], BF16)
                nc.vector.tensor_copy(out=agg_sb[:], in_=ps_agg[:])

                ps_t = psT.tile([128, D], BF16, space="PSUM")
                for qq in range(4):
                    sl = slice(qq * 128, (qq + 1) * 128)
                    nc.tensor.transpose(out=ps_t[:, sl], in_=agg_sb[:, sl], identity=ident[:])
                aggt = aggtp.tile([128, D], BF16)
                nc.vector.tensor_copy(out=aggt[:], in_=ps_t[:])

                ps_out = psO.tile([128, D], F32, space="PSUM")
                for qq in range(4):
                    sl = slice(qq * 128, (qq + 1) * 128)
                    nc.tensor.matmul(
                        out=ps_out[:], lhsT=aggt[:, sl], rhs=wt[:, qq, :],
                        start=(qq == 0), stop=False,
                    )
                nc.tensor.matmul(
                    out=ps_out[:], lhsT=ones1[:], rhs=bias[:],
                    start=False, stop=True,
                )

                xo = xop.tile([128, D], F32)
                nc.sync.dma_start(
                    out=xo[:rows], in_=xown_in[blk * BLK: blk * BLK + rows])
                t_res = resp.tile([128, D], F32)
                nc.scalar.activation(
                    out=t_res[:], in_=ps_out[:],
                    func=mybir.ActivationFunctionType.Relu,
                )
                nc.vector.tensor_add(out=t_res[:rows], in0=t_res[:rows], in1=xo[:rows])
                nc.sync.dma_start(
                    out=out[blk * BLK: blk * BLK + rows], in_=t_res[:rows])
    nc.finalize()
    return nc


def prepare(x, W, b, edge_weight, edge_src, edge_dst):
    """Returns (nc, in_maps) — the compiled-graph + per-core inputs."""
    x = np.asarray(x); W = np.asarray(W); b = np.asarray(b)
    ew = np.asarray(edge_weight); es = np.asarray(edge_src); ed = np.asarray(edge_dst)

    # capacity: max edges per (core, block, src-half), rounded to chunks
    core = ed // SHARD
    blk = (ed % SHARD) // BLK
    half = (es >= HALF).astype(np.int64)
    key = (core * N_BLK + blk) * 2 + half
    counts = np.bincount(key, minlength=N_CORES * N_BLK * 2)
    cpb = max(1, int(-(-counts.max() // BLK)))

    if cpb not in _CACHE:
        _CACHE[cpb] = _build(cpb)
    nc = _CACHE[cpb]

    xb = x.astype(BF)
    iota_np = np.tile(np.arange(BLK, dtype=np.float32), cpb)[None].repeat(128, 0).astype(BF)
    ident_np = np.eye(128, dtype=np.float32).astype(BF)
    common = {
        "xb_lo": xb[:HALF], "xb_hi": xb[HALF:],
        "w_in": W.astype(BF), "b_in": b.astype(BF)[None, :],
        "ident": ident_np, "iota": iota_np,
    }
    in_maps = []
    for c in range(N_CORES):
        srci, dcol, wv = _prep_core(c, es, ed, ew, cpb)
        in_maps.append({
            **common, "srci": srci, "dcol": dcol, "wv": wv,
            "xown": x[c * SHARD:(c + 1) * SHARD],
        })
    return nc, in_maps


def kernel(x, W, b, edge_weight, edge_src, edge_dst):
    from concourse.bass_utils import run_bass_kernel_spmd

    nc, in_maps = prepare(x, W, b, edge_weight, edge_src, edge_dst)
    res = run_bass_kernel_spmd(nc, in_maps, core_ids=list(range(N_CORES)))
    out = np.concatenate([res.results[c]["out"] for c in range(N_CORES)], axis=0)
    return out.astype(np.float32)
